# revision 20
# baseline (speedup 1.0000x reference)
"""Swin-3D window attention kernel for TRN2 (Bass/Tile), 8-core data parallel.

Problem: x[1,6,16,16,7,7,256] -> 256 windows of N=294 tokens, d=256.
Per window: qkv = x @ Wqkv.T; 8-head attention (dh=32) with relative-position
bias from a table; out proj. Data-parallel over windows: 32 windows/core.

Device-side layout (per window):
  xT    [256, 294]  (d on partitions, 2 chunks)             <- host pre-transposed
  qkT   [512, 294]  = (Wqk xT)                              (bf16 via psum copy)
  v     [n, (h,33)] = (xT.T Wv | ones)                      ones col 32 per head
  simT  [j, i] per head: lhsT=kT_h[dh, j-chunk], rhs=qT_h[dh, i]
  P     = exp(simT) * ebt  (ACT exp -> bf16, DVE/gpsimd bias mult)
  AVden [i-chunk, (h,33)] = P_h[j,ic].T @ [V_h|ones]        den = col 32
  avt   = AVden * recip(den)  (DVE bcast mult, psum->sbuf bf16)
  av    [he, i] via 6 PE transposes
  yT    [d, i] = Wout.T av   (2 d-chunks, accumulated over 2 he-chunks)
  host untransposes yT.

j is chunked [128, 102, 64]; windows are processed in PAIRS: the 64-row
j-tail of the even window sits at psum/sbuf partitions 0:64 and the odd
window's at 64:128, so ONE exp / bias-mult instruction covers both
windows' tails (the ACT exp stream is the kernel's bottleneck).

Scheduling notes (272052 -> 263129 ns TimelineSim):
 - heads of windows 0 AND 1 are prefetched before the ebt/wout const DMAs,
   and ebt is split into 3 DMAs so early xt loads aren't queued behind it;
 - several tail_ic / tail_transp callbacks moved from phase "mid" slots
   (between wv0 and wv1 of a sim phase) to "post" slots (after the whole
   phase) so prior-pair tail matmuls don't delay the wv1 exps;
 - the pair-tail bias multiply is split per 2-head quarter, each emitted
   right after the exp act that produces those heads, so downstream tail
   avdens are gated one act earlier (helps steady state AND the drain).
GPSIMD must not touch PSUM (walrus rejects it) - gpsimd ops here only
ever read/write SBUF. DMA cannot source from PSUM either.
"""

import numpy as np

import concourse.bass as bass
import concourse.mybir as mybir
import concourse.tile as tile
from concourse import bacc
from concourse.bass import ds, ts

F32 = mybir.dt.float32
BF16 = mybir.dt.bfloat16

AGENT, WIN = 6, 7
HEADS, DH = 8, 32
N = AGENT * WIN * WIN          # 294
D = 256
NB = 256                        # total windows
SCALE = DH ** -0.5
JC_SIZES = [128, 102, 64]       # j/n chunking of 294 (tail pair-shared)
J0 = [0, 128, 230]
IC_SIZES = [128, 128, 38]       # i chunking of 294


def rel_pos_index():
    coords = np.stack(np.meshgrid(np.arange(AGENT), np.arange(WIN), np.arange(WIN), indexing="ij"))
    flat = coords.reshape(3, -1)
    rel = flat[:, :, None] - flat[:, None, :]
    rel = rel.transpose(1, 2, 0).copy()
    rel[..., 0] += AGENT - 1
    rel[..., 1] += WIN - 1
    rel[..., 2] += WIN - 1
    rel[..., 0] *= (2 * WIN - 1) * (2 * WIN - 1)
    rel[..., 1] *= 2 * WIN - 1
    return rel.sum(-1)          # [N, N] int


def host_prep(x, w_qkv, w_out, bias_table, n_cores=8):
    """Full inputs -> per-core input maps (numpy only)."""
    import ml_dtypes

    W = NB // n_cores
    # x: [1,6,16,16,7,7,256] -> windows [B=256, n=294, d=256] -> xT [B, d, n]
    xw = np.ascontiguousarray(
        x.transpose(0, 2, 3, 1, 4, 5, 6).reshape(NB, N, D).transpose(0, 2, 1)
    )  # [256, 256, 294]

    wqkv_t = np.ascontiguousarray(w_qkv.T).copy()      # [256, 768] = [d, e]
    wqkv_t[:, :HEADS * DH] *= SCALE                    # fold q scale
    wqkv_t = wqkv_t.reshape(2, 128, 3 * HEADS * DH)    # d-chunked

    # wout for flipped proj: lhsT = w_out.T [e, d] chunked on e
    wout_t = np.ascontiguousarray(w_out.T).reshape(2, 128, D)  # [hc, 128(he), d]

    rpi = rel_pos_index()                              # [N(i), N(j)]
    bias = bias_table[rpi]                             # [i, j, h]
    ebt = np.exp(bias.transpose(1, 2, 0))              # [j, h, i]
    braw = bias.transpose(1, 2, 0)                     # [j, h, i] raw bias
    ebt_p = np.zeros((3, 128, HEADS * N), dtype=np.float32)
    for jc in range(2):
        jsz = JC_SIZES[jc]
        ebt_p[jc, :jsz, :] = ebt[J0[jc]:J0[jc] + jsz].reshape(jsz, HEADS * N)
    # jc0 heads 0-3: raw bias, added into sim psum by PE identity-matmuls
    ebt_p[0, :JC_SIZES[0], 0:4 * N] = braw[J0[0]:J0[0] + JC_SIZES[0], 0:4].reshape(JC_SIZES[0], 4 * N)
    tail = ebt[J0[2]:J0[2] + 64].reshape(64, HEADS * N)
    ebt_p[2, 0:64, :] = tail                           # even window's tail
    ebt_p[2, 64:128, :] = tail                         # odd window's tail (dup)

    ident = np.eye(128, dtype=np.float32)

    xw_bf16 = xw.astype(ml_dtypes.bfloat16)
    wqkv_bf16 = wqkv_t.astype(ml_dtypes.bfloat16)
    wout_bf16 = wout_t.astype(ml_dtypes.bfloat16)
    ebt_bf16 = ebt_p.astype(ml_dtypes.bfloat16)
    ident_bf16 = ident.astype(ml_dtypes.bfloat16)
    in_maps = []
    for c in range(n_cores):
        in_maps.append({
            "xt": np.ascontiguousarray(xw_bf16[c * W:(c + 1) * W]),
            "wqkv_t": wqkv_bf16,
            "wout_t": wout_bf16,
            "ebt": ebt_bf16,
            "ident": ident_bf16,
        })
    return in_maps


def host_assemble(results):
    """Per-core yT [W,2,128,294] -> full output [1,6,16,16,7,7,256]."""
    y_all = np.concatenate([r["y"] for r in results], axis=0)  # [256, 2, 128, 294]
    y_all = y_all.reshape(NB, D, N).transpose(0, 2, 1)         # [256, 294, 256]
    out = np.ascontiguousarray(y_all).reshape(1, 16, 16, AGENT, WIN, WIN, D)
    out = out.transpose(0, 3, 1, 2, 4, 5, 6)
    return np.ascontiguousarray(out)


class _St:
    """Per-window pipeline state."""
    __slots__ = ("w", "par", "qk", "v", "p", "recip", "avt", "av", "xt")

    def __init__(self, w, qk, recip, avt, av):
        self.w = w
        self.par = w % 2          # 0: tail at partitions 0:64, 1: at 64:128
        self.qk, self.v = qk, None
        self.p = []               # p tiles [jc0, jc1, pair-tail]
        self.recip, self.avt, self.av = recip, avt, av


def build_kernel(W=32, shared_bufs=2, dbg=False):
    nc = bacc.Bacc("TRN2", target_bir_lowering=False, debug=False)

    xt_d = nc.dram_tensor("xt", [W, D, N], BF16, kind="ExternalInput")
    wqkv_d = nc.dram_tensor("wqkv_t", [2, 128, 768], BF16, kind="ExternalInput")
    wout_d = nc.dram_tensor("wout_t", [2, 128, D], BF16, kind="ExternalInput")
    ebt_d = nc.dram_tensor("ebt", [3, 128, HEADS * N], BF16, kind="ExternalInput")
    ident_d = nc.dram_tensor("ident", [128, 128], BF16, kind="ExternalInput")
    y_d = nc.dram_tensor("y", [W, 2, 128, N], F32, kind="ExternalOutput")

    with tile.TileContext(nc) as tc:
        with (
            tc.tile_pool(name="const", bufs=1) as constp,
            tc.tile_pool(name="xt", bufs=3) as xtp,
            tc.tile_pool(name="qksb", bufs=4) as qkp,
            tc.tile_pool(name="vsb", bufs=4) as vp,
            tc.tile_pool(name="psb", bufs=12) as pp,
            tc.tile_pool(name="small", bufs=4) as smallp,
            tc.tile_pool(name="shared", bufs=3, space="PSUM") as ps_sh,
            tc.tile_pool(name="tail", bufs=2, space="PSUM") as ps_tail,
        ):
            # ---- persistent constants (ident/wqkv first: xt[0] and the PE
            # warmup depend on them; the big ebt transfer goes last) ----
            ident_sb = constp.tile([128, 128], BF16, tag="ident")
            nc.sync.dma_start(ident_sb[:], ident_d[:, :])
            wqkv_sb = constp.tile([128, 2, 768], BF16, tag="wqkv")
            nc.sync.dma_start(wqkv_sb[:], wqkv_d.rearrange("c p e -> p c e"))
            wout_sb = constp.tile([128, 2, D], BF16, tag="wout")
            ebt_sb = constp.tile([128, 3, HEADS * N], BF16, tag="ebt")

            def emit_head(w):
                """xt load + qk projections for window w -> (xt_sb, qk_sb)."""
                par = w % 2
                xt_sb = xtp.tile([128, 2, N], BF16, tag="xt")
                nc.sync.dma_start(xt_sb[:], xt_d[w].rearrange("(c p) n -> p c n", p=128))

                qk_sb = qkp.tile([128, 4, N], BF16, tag="qk")
                for eh in range(2):
                    qk_ps = ps_sh.tile([128, 2, 512], F32, tag="s", name="qk")
                    for ec2 in range(2):
                        ec = 2 * eh + ec2
                        for dc in range(2):
                            nc.tensor.matmul(
                                qk_ps[:, ec2, :N],
                                lhsT=wqkv_sb[:, dc, ts(ec, 128)],
                                rhs=xt_sb[:, dc, :],
                                start=(dc == 0), stop=(dc == 1),
                            )
                    if eh == 0:
                        nc.scalar.copy(qk_sb[:, 0:2, :], qk_ps[:, :, :N])
                    else:
                        nc.vector.tensor_copy(qk_sb[:, 2:4, :], qk_ps[:, :, :N])
                return xt_sb, qk_sb

            def emit_v(w, xt_sb):
                """V projection for window w (needed only a pair later).
                Odd windows put the 64-row n-tail of V at partitions 64:128."""
                po = 64 * (w % 2)
                v_ps = ps_sh.tile([128, 2, 512], F32, tag="s", name="v")
                v_mm = [
                    (0, 0, 128, 0, 0),      # (nc2, psum-part-base, nsz, bank, off)
                    (1, 0, 102, 0, 256),
                    (2, po, 64, 1, 0),
                ]
                for nc2, pb, nsz, bank, off in v_mm:
                    for dc in range(2):
                        nc.tensor.matmul(
                            v_ps[pb:pb + nsz, bank, off:off + 256],
                            lhsT=xt_sb[:, dc, ds(J0[nc2], nsz)],
                            rhs=wqkv_sb[:, dc, 512:768],
                            start=(dc == 0), stop=(dc == 1),
                            tile_position=(0, pb),
                        )
                v_sb = vp.tile([128, 3, HEADS * 33], BF16, tag="v")
                v_view = v_sb[:].rearrange("p c (h e) -> p c h e", e=33)
                nc.vector.memset(v_view[:, :, :, 32], 1.0)
                nc.vector.tensor_copy(
                    v_view[:, 0:2, :, 0:32],
                    v_ps[:, 0, :].rearrange("p (c h e) -> p c h e", c=2, e=32),
                )
                nc.vector.tensor_copy(
                    v_view[po:po + 64, 2, :, 0:32],
                    v_ps[po:po + 64, 1, 0:256].rearrange("p (h e) -> p h e", e=32),
                )
                return v_sb

            def new_state(w, head):
                xt_sb, qk_sb = head
                recip_sb = smallp.tile([128, 3, 8], F32, tag="recip")
                avt_sb = smallp.tile([128, 3, 256], BF16, tag="avtsb")
                av_sb = smallp.tile([128, 2, 304], BF16, tag="av")
                st = _St(w, qk_sb, recip_sb, avt_sb, av_sb)
                st.xt = xt_sb
                return st

            def emit_simjc(st, jc, mid=None, post=None):
                """One non-tail j-chunk: sim mms, exps, bias mult."""
                jsz = JC_SIZES[jc]
                qk_sb = st.qk
                p_sb = pp.tile([128, HEADS * N], BF16, tag="p")
                st.p.append(p_sb)
                for wv in range(2):
                    if wv == 1 and mid is not None:
                        for fn in mid:
                            fn()
                    tiles = [
                        ps_sh.tile([128, 2, 512], F32, tag="s", name="sim_a"),
                        ps_sh.tile([128, 2, 512], F32, tag="s", name="sim_b"),
                    ]
                    for b4 in range(4):
                        h = 4 * wv + b4
                        kec, kpp = 2 + h // 4, 32 * (h % 4)
                        qec, qpp = h // 4, 32 * (h % 4)
                        badd = jc == 0 and h < 4
                        nc.tensor.matmul(
                            tiles[b4 // 2][:jsz, b4 % 2, :N],
                            lhsT=qk_sb[kpp:kpp + 32, kec, ds(J0[jc], jsz)],
                            rhs=qk_sb[qpp:qpp + 32, qec, :],
                            start=True, stop=not badd,
                            tile_position=(32 * (h % 4), 0),
                        )
                        if badd:
                            # += raw bias via identity matmul: exp(sim + b)
                            nc.tensor.matmul(
                                tiles[b4 // 2][:jsz, b4 % 2, :N],
                                lhsT=ident_sb[:jsz, :jsz],
                                rhs=ebt_sb[:jsz, 0, ds(h * N, N)],
                                start=False, stop=True,
                            )
                    for t in range(2):
                        g = 2 * wv + t
                        nc.scalar.activation(
                            p_sb[:jsz, 2 * g * N:(2 * g + 2) * N].rearrange("p (c n) -> p c n", n=N),
                            tiles[t][:jsz, :, :N],
                            mybir.ActivationFunctionType.Exp,
                        )
                if post is not None:
                    for fn in post:
                        fn()
                # bias multiply: jc0 on gpsimd (2 halves), jc1 on DVE
                if jc == 0:
                    nc.gpsimd.tensor_mul(
                        p_sb[:jsz, ds(4 * N, 4 * N)],
                        p_sb[:jsz, ds(4 * N, 4 * N)],
                        ebt_sb[:jsz, jc, ds(4 * N, 4 * N)],
                    )
                else:
                    # jc1: front half DVE, back half gpsimd (consumed 1.5
                    # phases later, so the slower engine has slack)
                    nc.vector.tensor_mul(
                        p_sb[:jsz, ds(0, 4 * N)],
                        p_sb[:jsz, ds(0, 4 * N)],
                        ebt_sb[:jsz, jc, ds(0, 4 * N)],
                    )
                    nc.gpsimd.tensor_mul(
                        p_sb[:jsz, ds(4 * N, 4 * N)],
                        p_sb[:jsz, ds(4 * N, 4 * N)],
                        ebt_sb[:jsz, jc, ds(4 * N, 4 * N)],
                    )

            def emit_pairtail(stA, stB, mid=None, post=None):
                """Shared j-tail of a window pair: A at partitions 0:64,
                B at 64:128; one exp / bias-mult instruction covers both."""
                p_sb = pp.tile([128, HEADS * N], BF16, tag="p")
                stA.p.append(p_sb)
                stB.p.append(p_sb)
                for wv in range(2):
                    if wv == 1 and mid is not None:
                        for fn in mid:
                            fn()
                    tiles = [
                        ps_sh.tile([128, 2, 512], F32, tag="s", name="tsim_a"),
                        ps_sh.tile([128, 2, 512], F32, tag="s", name="tsim_b"),
                    ]
                    for b4 in range(4):
                        h = 4 * wv + b4
                        kec, kpp = 2 + h // 4, 32 * (h % 4)
                        qec, qpp = h // 4, 32 * (h % 4)
                        for po, stX in ((0, stA), (64, stB)):
                            nc.tensor.matmul(
                                tiles[b4 // 2][po:po + 64, b4 % 2, :N],
                                lhsT=stX.qk[kpp:kpp + 32, kec, ds(J0[2], 64)],
                                rhs=stX.qk[qpp:qpp + 32, qec, :],
                                start=True, stop=True,
                                tile_position=(32 * (h % 4), po),
                            )
                    for t in range(2):
                        g = 2 * wv + t
                        nc.scalar.activation(
                            p_sb[:, 2 * g * N:(2 * g + 2) * N].rearrange("p (c n) -> p c n", n=N),
                            tiles[t][:, :, :N],
                            mybir.ActivationFunctionType.Exp,
                        )
                        # per-quarter bias mult: fires as soon as this act's
                        # two heads land, so the tail avden gate moves earlier
                        nc.vector.tensor_mul(
                            p_sb[:, ds(2 * g * N, 2 * N)], p_sb[:, ds(2 * g * N, 2 * N)],
                            ebt_sb[:, 2, ds(2 * g * N, 2 * N)],
                        )
                if post is not None:
                    for fn in post:
                        fn()

            def tail_avden(st, ics, avt_ps):
                """AVden matmuls for i-chunks `ics` of a finished window.
                out[i, (h,33)]: col 32 of each head = softmax denominator."""
                po = 64 * st.par
                for slot, ic in enumerate(ics):
                    icsz = IC_SIZES[ic]
                    for h in range(HEADS):
                        for jc in range(3):
                            jsz = JC_SIZES[jc]
                            jb = po if jc == 2 else 0
                            nc.tensor.matmul(
                                avt_ps[:icsz, slot, 33 * h:33 * h + 33],
                                lhsT=st.p[jc][jb:jb + jsz, ds(h * N + 128 * ic, icsz)],
                                rhs=st.v[jb:jb + jsz, jc, 33 * h:33 * h + 33],
                                start=(jc == 0), stop=(jc == 2),
                                skip_group_check=True,
                                tile_position=(jb, 0),
                            )

            def tail_norm(st, ics, avt_ps, eng=None):
                """recip(den) + normalize AVden into sbuf bf16 [128, 3, 256]."""
                for slot, ic in enumerate(ics):
                    icsz = IC_SIZES[ic]
                    nc.vector.reciprocal_approx_fast(
                        out=st.recip[:icsz, ic, :],
                        in_=avt_ps[:icsz, slot, 0:33 * HEADS].rearrange("p (h e) -> p h e", e=33)[:, :, 32],
                    )
                    (eng or nc.vector).tensor_mul(
                        st.avt[:icsz, ic, :].rearrange("p (h e) -> p h e", e=32),
                        avt_ps[:icsz, slot, 0:33 * HEADS].rearrange("p (h e) -> p h e", e=33)[:, :, 0:32],
                        st.recip[:icsz, ic, :].broadcast_to([icsz, HEADS, 32]),
                    )

            def tail_transp(st):
                """Transpose avt -> av via 6 PE transposes."""
                avtr_ps = ps_tail.tile([128, 2, N], BF16, tag="t", name="avtr")
                for ic, icsz in enumerate(IC_SIZES):
                    for hf in range(2):
                        nc.tensor.transpose(
                            avtr_ps[:, hf, ds(128 * ic, icsz)],
                            st.avt[:icsz, ic, ts(hf, 128)],
                            ident_sb[:icsz, :icsz],
                        )
                nc.vector.tensor_copy(st.av[:, :, :N], avtr_ps[:])

            def tail_proj(st):
                """Flipped out-projection, store yT."""
                y_sb = smallp.tile([128, 2, N], F32, tag="y")
                for dc2 in range(2):
                    yt_ps = ps_tail.tile([128, 512], F32, tag="t", name="yt")
                    for hc in range(2):
                        nc.tensor.matmul(
                            yt_ps[:, :N],
                            lhsT=wout_sb[:, hc, ts(dc2, 128)],
                            rhs=st.av[:, hc, :N],
                            start=(hc == 0), stop=(hc == 1),
                        )
                    nc.vector.tensor_copy(y_sb[:, dc2, :], yt_ps[:, :N])
                nc.sync.dma_start(
                    y_d[st.w, :, :, :].rearrange("c p n -> p c n"),
                    y_sb[:],
                )

            def tail_ic(st, ic):
                avt = ps_tail.tile([128, 1, 512], F32, tag="t", name="avt%d" % ic)
                tail_avden(st, [ic], avt)
                tail_norm(st, [ic], avt)

            # ---- pair-pipelined main loop ----
            pA = pB = None            # previous pair's states
            # warm the PE p-state ramp while the first xt DMA is in flight
            warm_ps = ps_sh.tile([128, 2, 512], F32, tag="s", name="warm")
            for _ in range(16):
                nc.tensor.matmul(
                    warm_ps[:, 0, :128], lhsT=ident_sb[:], rhs=ident_sb[:],
                    start=True, stop=True, skip_group_check=True,
                )
            head = emit_head(0)
            # prefetch window 1's head too, and split the big ebt transfer so
            # later xt DMAs are not queued behind one 5us DMA
            nh0 = emit_head(1)
            for _c in range(3):
                nc.sync.dma_start(ebt_sb[:, _c, :], ebt_d[_c])
            nc.sync.dma_start(wout_sb[:], wout_d.rearrange("c p e -> p c e"))
            for k in range(W // 2):
                w0, w1 = 2 * k, 2 * k + 1
                nh = {}
                stA = new_state(w0, head)
                mid = [] if k == 0 else [lambda: nh.__setitem__("B", emit_head(w1))]
                emit_simjc(stA, 0, mid,
                           post=[lambda: tail_ic(pA, 0)] if pA is not None else None)  # P0
                if pA is not None:
                    tail_ic(pA, 1)
                mid = [lambda: tail_ic(pA, 2)] if pA is not None else []
                mid.append(lambda: stA.__setattr__("v", emit_v(w0, stA.xt)))
                emit_simjc(stA, 1, mid)               # P1
                stB = new_state(w1, nh0 if k == 0 else nh["B"])
                mid = [lambda: tail_transp(pA)] if pA is not None else []
                if k + 1 < W // 2:
                    mid.append(lambda: nh.__setitem__("A2", emit_head(2 * k + 2)))
                emit_simjc(stB, 0, mid,
                           post=[lambda: tail_ic(pB, 0)] if pB is not None else None)  # P2
                if pA is not None:
                    tail_proj(pA)
                mid = [lambda: stB.__setattr__("v", emit_v(w1, stB.xt))]
                emit_simjc(stB, 1, mid,
                           post=[lambda: tail_ic(pB, 1)] if pB is not None else None)  # P3
                mid = [lambda: tail_ic(pB, 2)] if pB is not None else None
                emit_pairtail(stA, stB, mid,
                              post=[lambda: tail_transp(pB)] if pB is not None else None)  # P4
                if pB is not None:
                    tail_proj(pB)
                head = nh.get("A2")
                pA, pB = stA, stB

            # epilogue: interleave the two windows' tails (independent)
            for ic in range(3):
                avtA = ps_tail.tile([128, 1, 512], F32, tag="t", name="eavtA")
                tail_avden(pA, [ic], avtA)
                avtB = ps_sh.tile([128, 2, 512], F32, tag="s", name="eavtB")
                tail_avden(pB, [ic], avtB)
                tail_norm(pA, [ic], avtA)
                tail_norm(pB, [ic], avtB)
            tail_transp(pA)
            tail_transp(pB)
            tail_proj(pA)
            tail_proj(pB)

    nc.finalize()
    return nc


# ---------------------------------------------------------------------------
# Harness entry point: full inputs in, full output out. Shards the 256
# windows across 8 NeuronCores (32 each), runs the Bass kernel via
# run_bass_kernel_spmd, and reassembles the full output.
# ---------------------------------------------------------------------------
from concourse.bass_utils import run_bass_kernel_spmd

_NC_CACHE = {}


def _get_nc():
    if "nc" not in _NC_CACHE:
        _NC_CACHE["nc"] = build_kernel(W=NB // 8)
    return _NC_CACHE["nc"]


def kernel(x, w_qkv, w_out, bias_table):
    x = np.asarray(x, dtype=np.float32)
    w_qkv = np.asarray(w_qkv, dtype=np.float32)
    w_out = np.asarray(w_out, dtype=np.float32)
    bias_table = np.asarray(bias_table, dtype=np.float32)

    in_maps = host_prep(x, w_qkv, w_out, bias_table, n_cores=8)
    nc = _get_nc()
    res = run_bass_kernel_spmd(nc, in_maps, core_ids=list(range(8)))
    return host_assemble(res.results)



# revision 21
# speedup vs baseline: 1.0014x; 1.0014x over previous
"""Swin-3D window attention kernel for TRN2 (Bass/Tile), 8-core data parallel.

Problem: x[1,6,16,16,7,7,256] -> 256 windows of N=294 tokens, d=256.
Per window: qkv = x @ Wqkv.T; 8-head attention (dh=32) with relative-position
bias from a table; out proj. Data-parallel over windows: 32 windows/core.

Device-side layout (per window):
  xT    [256, 294]  (d on partitions, 2 chunks)             <- host pre-transposed
  qkT   [512, 294]  = (Wqk xT)                              (bf16 via psum copy)
  v     [n, (h,33)] = (xT.T Wv | ones)                      ones col 32 per head
  simT  [j, i] per head: lhsT=kT_h[dh, j-chunk], rhs=qT_h[dh, i]
  P     = exp(simT) * ebt  (ACT exp -> bf16, DVE/gpsimd bias mult)
  AVden [i-chunk, (h,33)] = P_h[j,ic].T @ [V_h|ones]        den = col 32
  avt   = AVden * recip(den)  (DVE bcast mult, psum->sbuf bf16)
  av    [he, i] via 6 PE transposes
  yT    [d, i] = Wout.T av   (2 d-chunks, accumulated over 2 he-chunks)
  host untransposes yT.

j is chunked [128, 102, 64]; windows are processed in PAIRS: the 64-row
j-tail of the even window sits at psum/sbuf partitions 0:64 and the odd
window's at 64:128, so ONE exp / bias-mult instruction covers both
windows' tails (the ACT exp stream is the kernel's bottleneck).

Scheduling notes (272052 -> 263129 ns TimelineSim):
 - heads of windows 0 AND 1 are prefetched before the ebt/wout const DMAs,
   and ebt is split into 3 DMAs so early xt loads aren't queued behind it;
 - several tail_ic / tail_transp callbacks moved from phase "mid" slots
   (between wv0 and wv1 of a sim phase) to "post" slots (after the whole
   phase) so prior-pair tail matmuls don't delay the wv1 exps;
 - the pair-tail bias multiply is split per 2-head quarter, each emitted
   right after the exp act that produces those heads, so downstream tail
   avdens are gated one act earlier (helps steady state AND the drain).
GPSIMD must not touch PSUM (walrus rejects it) - gpsimd ops here only
ever read/write SBUF. DMA cannot source from PSUM either.
"""

import numpy as np

import concourse.bass as bass
import concourse.mybir as mybir
import concourse.tile as tile
from concourse import bacc
from concourse.bass import ds, ts

F32 = mybir.dt.float32
BF16 = mybir.dt.bfloat16

AGENT, WIN = 6, 7
HEADS, DH = 8, 32
N = AGENT * WIN * WIN          # 294
D = 256
NB = 256                        # total windows
SCALE = DH ** -0.5
JC_SIZES = [128, 102, 64]       # j/n chunking of 294 (tail pair-shared)
J0 = [0, 128, 230]
IC_SIZES = [128, 128, 38]       # i chunking of 294


def rel_pos_index():
    coords = np.stack(np.meshgrid(np.arange(AGENT), np.arange(WIN), np.arange(WIN), indexing="ij"))
    flat = coords.reshape(3, -1)
    rel = flat[:, :, None] - flat[:, None, :]
    rel = rel.transpose(1, 2, 0).copy()
    rel[..., 0] += AGENT - 1
    rel[..., 1] += WIN - 1
    rel[..., 2] += WIN - 1
    rel[..., 0] *= (2 * WIN - 1) * (2 * WIN - 1)
    rel[..., 1] *= 2 * WIN - 1
    return rel.sum(-1)          # [N, N] int


def host_prep(x, w_qkv, w_out, bias_table, n_cores=8):
    """Full inputs -> per-core input maps (numpy only)."""
    import ml_dtypes

    W = NB // n_cores
    # x: [1,6,16,16,7,7,256] -> windows [B=256, n=294, d=256] -> xT [B, d, n]
    xw = np.ascontiguousarray(
        x.transpose(0, 2, 3, 1, 4, 5, 6).reshape(NB, N, D).transpose(0, 2, 1)
    )  # [256, 256, 294]

    wqkv_t = np.ascontiguousarray(w_qkv.T).copy()      # [256, 768] = [d, e]
    wqkv_t[:, :HEADS * DH] *= SCALE                    # fold q scale
    wqkv_t = wqkv_t.reshape(2, 128, 3 * HEADS * DH)    # d-chunked

    # wout for flipped proj: lhsT = w_out.T [e, d] chunked on e
    wout_t = np.ascontiguousarray(w_out.T).reshape(2, 128, D)  # [hc, 128(he), d]

    rpi = rel_pos_index()                              # [N(i), N(j)]
    bias = bias_table[rpi]                             # [i, j, h]
    ebt = np.exp(bias.transpose(1, 2, 0))              # [j, h, i]
    braw = bias.transpose(1, 2, 0)                     # [j, h, i] raw bias
    ebt_p = np.zeros((3, 128, HEADS * N), dtype=np.float32)
    for jc in range(2):
        jsz = JC_SIZES[jc]
        ebt_p[jc, :jsz, :] = ebt[J0[jc]:J0[jc] + jsz].reshape(jsz, HEADS * N)
    # jc0 heads 0-3: raw bias, added into sim psum by PE identity-matmuls
    ebt_p[0, :JC_SIZES[0], 0:4 * N] = braw[J0[0]:J0[0] + JC_SIZES[0], 0:4].reshape(JC_SIZES[0], 4 * N)
    tail = ebt[J0[2]:J0[2] + 64].reshape(64, HEADS * N)
    ebt_p[2, 0:64, :] = tail                           # even window's tail
    ebt_p[2, 64:128, :] = tail                         # odd window's tail (dup)

    ident = np.eye(128, dtype=np.float32)

    xw_bf16 = xw.astype(ml_dtypes.bfloat16)
    wqkv_bf16 = wqkv_t.astype(ml_dtypes.bfloat16)
    wout_bf16 = wout_t.astype(ml_dtypes.bfloat16)
    ebt_bf16 = ebt_p.astype(ml_dtypes.bfloat16)
    ident_bf16 = ident.astype(ml_dtypes.bfloat16)
    in_maps = []
    for c in range(n_cores):
        in_maps.append({
            "xt": np.ascontiguousarray(xw_bf16[c * W:(c + 1) * W]),
            "wqkv_t": wqkv_bf16,
            "wout_t": wout_bf16,
            "ebt": ebt_bf16,
            "ident": ident_bf16,
        })
    return in_maps


def host_assemble(results):
    """Per-core yT [W,2,128,294] -> full output [1,6,16,16,7,7,256]."""
    y_all = np.concatenate([r["y"] for r in results], axis=0)  # [256, 2, 128, 294]
    y_all = y_all.reshape(NB, D, N).transpose(0, 2, 1)         # [256, 294, 256]
    out = np.ascontiguousarray(y_all).reshape(1, 16, 16, AGENT, WIN, WIN, D)
    out = out.transpose(0, 3, 1, 2, 4, 5, 6)
    return np.ascontiguousarray(out)


class _St:
    """Per-window pipeline state."""
    __slots__ = ("w", "par", "qk", "v", "p", "recip", "avt", "av", "xt")

    def __init__(self, w, qk, recip, avt, av):
        self.w = w
        self.par = w % 2          # 0: tail at partitions 0:64, 1: at 64:128
        self.qk, self.v = qk, None
        self.p = []               # p tiles [jc0, jc1, pair-tail]
        self.recip, self.avt, self.av = recip, avt, av


def build_kernel(W=32, shared_bufs=2, dbg=False):
    nc = bacc.Bacc("TRN2", target_bir_lowering=False, debug=False)

    xt_d = nc.dram_tensor("xt", [W, D, N], BF16, kind="ExternalInput")
    wqkv_d = nc.dram_tensor("wqkv_t", [2, 128, 768], BF16, kind="ExternalInput")
    wout_d = nc.dram_tensor("wout_t", [2, 128, D], BF16, kind="ExternalInput")
    ebt_d = nc.dram_tensor("ebt", [3, 128, HEADS * N], BF16, kind="ExternalInput")
    ident_d = nc.dram_tensor("ident", [128, 128], BF16, kind="ExternalInput")
    y_d = nc.dram_tensor("y", [W, 2, 128, N], F32, kind="ExternalOutput")

    with tile.TileContext(nc) as tc:
        with (
            tc.tile_pool(name="const", bufs=1) as constp,
            tc.tile_pool(name="xt", bufs=3) as xtp,
            tc.tile_pool(name="qksb", bufs=4) as qkp,
            tc.tile_pool(name="vsb", bufs=4) as vp,
            tc.tile_pool(name="psb", bufs=15) as pp,
            tc.tile_pool(name="small", bufs=4) as smallp,
            tc.tile_pool(name="shared", bufs=3, space="PSUM") as ps_sh,
            tc.tile_pool(name="tail", bufs=2, space="PSUM") as ps_tail,
        ):
            # ---- persistent constants (ident/wqkv first: xt[0] and the PE
            # warmup depend on them; the big ebt transfer goes last) ----
            ident_sb = constp.tile([128, 128], BF16, tag="ident")
            nc.sync.dma_start(ident_sb[:], ident_d[:, :])
            wqkv_sb = constp.tile([128, 2, 768], BF16, tag="wqkv")
            nc.sync.dma_start(wqkv_sb[:], wqkv_d.rearrange("c p e -> p c e"))
            wout_sb = constp.tile([128, 2, D], BF16, tag="wout")
            ebt_sb = constp.tile([128, 3, HEADS * N], BF16, tag="ebt")

            def emit_head(w):
                """xt load + qk projections for window w -> (xt_sb, qk_sb)."""
                par = w % 2
                xt_sb = xtp.tile([128, 2, N], BF16, tag="xt")
                nc.sync.dma_start(xt_sb[:], xt_d[w].rearrange("(c p) n -> p c n", p=128))

                qk_sb = qkp.tile([128, 4, N], BF16, tag="qk")
                for eh in range(2):
                    qk_ps = ps_sh.tile([128, 2, 512], F32, tag="s", name="qk")
                    for ec2 in range(2):
                        ec = 2 * eh + ec2
                        for dc in range(2):
                            nc.tensor.matmul(
                                qk_ps[:, ec2, :N],
                                lhsT=wqkv_sb[:, dc, ts(ec, 128)],
                                rhs=xt_sb[:, dc, :],
                                start=(dc == 0), stop=(dc == 1),
                            )
                    if eh == 0:
                        nc.scalar.copy(qk_sb[:, 0:2, :], qk_ps[:, :, :N])
                    else:
                        nc.vector.tensor_copy(qk_sb[:, 2:4, :], qk_ps[:, :, :N])
                return xt_sb, qk_sb

            def emit_v(w, xt_sb):
                """V projection for window w (needed only a pair later).
                Odd windows put the 64-row n-tail of V at partitions 64:128."""
                po = 64 * (w % 2)
                v_ps = ps_sh.tile([128, 2, 512], F32, tag="s", name="v")
                v_mm = [
                    (0, 0, 128, 0, 0),      # (nc2, psum-part-base, nsz, bank, off)
                    (1, 0, 102, 0, 256),
                    (2, po, 64, 1, 0),
                ]
                for nc2, pb, nsz, bank, off in v_mm:
                    for dc in range(2):
                        nc.tensor.matmul(
                            v_ps[pb:pb + nsz, bank, off:off + 256],
                            lhsT=xt_sb[:, dc, ds(J0[nc2], nsz)],
                            rhs=wqkv_sb[:, dc, 512:768],
                            start=(dc == 0), stop=(dc == 1),
                            tile_position=(0, pb),
                        )
                v_sb = vp.tile([128, 3, HEADS * 33], BF16, tag="v")
                v_view = v_sb[:].rearrange("p c (h e) -> p c h e", e=33)
                nc.vector.memset(v_view[:, :, :, 32], 1.0)
                nc.vector.tensor_copy(
                    v_view[:, 0:2, :, 0:32],
                    v_ps[:, 0, :].rearrange("p (c h e) -> p c h e", c=2, e=32),
                )
                nc.vector.tensor_copy(
                    v_view[po:po + 64, 2, :, 0:32],
                    v_ps[po:po + 64, 1, 0:256].rearrange("p (h e) -> p h e", e=32),
                )
                return v_sb

            def new_state(w, head):
                xt_sb, qk_sb = head
                recip_sb = smallp.tile([128, 3, 8], F32, tag="recip")
                avt_sb = smallp.tile([128, 3, 256], BF16, tag="avtsb")
                av_sb = smallp.tile([128, 2, 304], BF16, tag="av")
                st = _St(w, qk_sb, recip_sb, avt_sb, av_sb)
                st.xt = xt_sb
                return st

            def emit_simjc(st, jc, mid=None, post=None):
                """One non-tail j-chunk: sim mms, exps, bias mult."""
                jsz = JC_SIZES[jc]
                qk_sb = st.qk
                p_sb = pp.tile([128, HEADS * N], BF16, tag="p")
                st.p.append(p_sb)
                for wv in range(2):
                    if wv == 1 and mid is not None:
                        for fn in mid:
                            fn()
                    tiles = [
                        ps_sh.tile([128, 2, 512], F32, tag="s", name="sim_a"),
                        ps_sh.tile([128, 2, 512], F32, tag="s", name="sim_b"),
                    ]
                    for b4 in range(4):
                        h = 4 * wv + b4
                        kec, kpp = 2 + h // 4, 32 * (h % 4)
                        qec, qpp = h // 4, 32 * (h % 4)
                        badd = jc == 0 and h < 4
                        nc.tensor.matmul(
                            tiles[b4 // 2][:jsz, b4 % 2, :N],
                            lhsT=qk_sb[kpp:kpp + 32, kec, ds(J0[jc], jsz)],
                            rhs=qk_sb[qpp:qpp + 32, qec, :],
                            start=True, stop=not badd,
                            tile_position=(32 * (h % 4), 0),
                        )
                        if badd:
                            # += raw bias via identity matmul: exp(sim + b)
                            nc.tensor.matmul(
                                tiles[b4 // 2][:jsz, b4 % 2, :N],
                                lhsT=ident_sb[:jsz, :jsz],
                                rhs=ebt_sb[:jsz, 0, ds(h * N, N)],
                                start=False, stop=True,
                            )
                    for t in range(2):
                        g = 2 * wv + t
                        nc.scalar.activation(
                            p_sb[:jsz, 2 * g * N:(2 * g + 2) * N].rearrange("p (c n) -> p c n", n=N),
                            tiles[t][:jsz, :, :N],
                            mybir.ActivationFunctionType.Exp,
                        )
                if post is not None:
                    for fn in post:
                        fn()
                # bias multiply: jc0 on gpsimd (2 halves), jc1 on DVE
                if jc == 0:
                    nc.gpsimd.tensor_mul(
                        p_sb[:jsz, ds(4 * N, 4 * N)],
                        p_sb[:jsz, ds(4 * N, 4 * N)],
                        ebt_sb[:jsz, jc, ds(4 * N, 4 * N)],
                    )
                else:
                    # jc1: front half DVE, back half gpsimd (consumed 1.5
                    # phases later, so the slower engine has slack)
                    nc.vector.tensor_mul(
                        p_sb[:jsz, ds(0, 4 * N)],
                        p_sb[:jsz, ds(0, 4 * N)],
                        ebt_sb[:jsz, jc, ds(0, 4 * N)],
                    )
                    nc.gpsimd.tensor_mul(
                        p_sb[:jsz, ds(4 * N, 4 * N)],
                        p_sb[:jsz, ds(4 * N, 4 * N)],
                        ebt_sb[:jsz, jc, ds(4 * N, 4 * N)],
                    )

            def emit_pairtail(stA, stB, mid=None, post=None):
                """Shared j-tail of a window pair: A at partitions 0:64,
                B at 64:128; one exp / bias-mult instruction covers both."""
                p_sb = pp.tile([128, HEADS * N], BF16, tag="p")
                stA.p.append(p_sb)
                stB.p.append(p_sb)
                for wv in range(2):
                    if wv == 1 and mid is not None:
                        for fn in mid:
                            fn()
                    tiles = [
                        ps_sh.tile([128, 2, 512], F32, tag="s", name="tsim_a"),
                        ps_sh.tile([128, 2, 512], F32, tag="s", name="tsim_b"),
                    ]
                    for b4 in range(4):
                        h = 4 * wv + b4
                        kec, kpp = 2 + h // 4, 32 * (h % 4)
                        qec, qpp = h // 4, 32 * (h % 4)
                        for po, stX in ((0, stA), (64, stB)):
                            nc.tensor.matmul(
                                tiles[b4 // 2][po:po + 64, b4 % 2, :N],
                                lhsT=stX.qk[kpp:kpp + 32, kec, ds(J0[2], 64)],
                                rhs=stX.qk[qpp:qpp + 32, qec, :],
                                start=True, stop=True,
                                tile_position=(32 * (h % 4), po),
                            )
                    for t in range(2):
                        g = 2 * wv + t
                        nc.scalar.activation(
                            p_sb[:, 2 * g * N:(2 * g + 2) * N].rearrange("p (c n) -> p c n", n=N),
                            tiles[t][:, :, :N],
                            mybir.ActivationFunctionType.Exp,
                        )
                        # per-quarter bias mult: fires as soon as this act's
                        # two heads land, so the tail avden gate moves earlier
                        nc.vector.tensor_mul(
                            p_sb[:, ds(2 * g * N, 2 * N)], p_sb[:, ds(2 * g * N, 2 * N)],
                            ebt_sb[:, 2, ds(2 * g * N, 2 * N)],
                        )
                if post is not None:
                    for fn in post:
                        fn()

            def tail_avden(st, ics, avt_ps):
                """AVden matmuls for i-chunks `ics` of a finished window.
                out[i, (h,33)]: col 32 of each head = softmax denominator."""
                po = 64 * st.par
                for slot, ic in enumerate(ics):
                    icsz = IC_SIZES[ic]
                    for h in range(HEADS):
                        for jc in range(3):
                            jsz = JC_SIZES[jc]
                            jb = po if jc == 2 else 0
                            nc.tensor.matmul(
                                avt_ps[:icsz, slot, 33 * h:33 * h + 33],
                                lhsT=st.p[jc][jb:jb + jsz, ds(h * N + 128 * ic, icsz)],
                                rhs=st.v[jb:jb + jsz, jc, 33 * h:33 * h + 33],
                                start=(jc == 0), stop=(jc == 2),
                                skip_group_check=True,
                                tile_position=(jb, 0),
                            )

            def tail_norm(st, ics, avt_ps, eng=None):
                """recip(den) + normalize AVden into sbuf bf16 [128, 3, 256]."""
                for slot, ic in enumerate(ics):
                    icsz = IC_SIZES[ic]
                    nc.vector.reciprocal_approx_fast(
                        out=st.recip[:icsz, ic, :],
                        in_=avt_ps[:icsz, slot, 0:33 * HEADS].rearrange("p (h e) -> p h e", e=33)[:, :, 32],
                    )
                    (eng or nc.vector).tensor_mul(
                        st.avt[:icsz, ic, :].rearrange("p (h e) -> p h e", e=32),
                        avt_ps[:icsz, slot, 0:33 * HEADS].rearrange("p (h e) -> p h e", e=33)[:, :, 0:32],
                        st.recip[:icsz, ic, :].broadcast_to([icsz, HEADS, 32]),
                    )

            def tail_transp(st):
                """Transpose avt -> av via 6 PE transposes."""
                avtr_ps = ps_tail.tile([128, 2, N], BF16, tag="t", name="avtr")
                for ic, icsz in enumerate(IC_SIZES):
                    for hf in range(2):
                        nc.tensor.transpose(
                            avtr_ps[:, hf, ds(128 * ic, icsz)],
                            st.avt[:icsz, ic, ts(hf, 128)],
                            ident_sb[:icsz, :icsz],
                        )
                nc.vector.tensor_copy(st.av[:, :, :N], avtr_ps[:])

            def tail_proj(st):
                """Flipped out-projection, store yT."""
                y_sb = smallp.tile([128, 2, N], F32, tag="y")
                for dc2 in range(2):
                    yt_ps = ps_tail.tile([128, 512], F32, tag="t", name="yt")
                    for hc in range(2):
                        nc.tensor.matmul(
                            yt_ps[:, :N],
                            lhsT=wout_sb[:, hc, ts(dc2, 128)],
                            rhs=st.av[:, hc, :N],
                            start=(hc == 0), stop=(hc == 1),
                        )
                    nc.vector.tensor_copy(y_sb[:, dc2, :], yt_ps[:, :N])
                nc.sync.dma_start(
                    y_d[st.w, :, :, :].rearrange("c p n -> p c n"),
                    y_sb[:],
                )

            def tail_ic(st, ic):
                avt = ps_tail.tile([128, 1, 512], F32, tag="t", name="avt%d" % ic)
                tail_avden(st, [ic], avt)
                tail_norm(st, [ic], avt)

            # ---- pair-pipelined main loop ----
            pA = pB = None            # previous pair's states
            # warm the PE p-state ramp while the first xt DMA is in flight
            warm_ps = ps_sh.tile([128, 2, 512], F32, tag="s", name="warm")
            for _ in range(16):
                nc.tensor.matmul(
                    warm_ps[:, 0, :128], lhsT=ident_sb[:], rhs=ident_sb[:],
                    start=True, stop=True, skip_group_check=True,
                )
            head = emit_head(0)
            # prefetch window 1's head too, and split the big ebt transfer so
            # later xt DMAs are not queued behind one 5us DMA
            nh0 = emit_head(1)
            for _c in range(3):
                nc.sync.dma_start(ebt_sb[:, _c, :], ebt_d[_c])
            nc.sync.dma_start(wout_sb[:], wout_d.rearrange("c p e -> p c e"))
            for k in range(W // 2):
                w0, w1 = 2 * k, 2 * k + 1
                nh = {}
                stA = new_state(w0, head)
                mid = [] if k == 0 else [lambda: nh.__setitem__("B", emit_head(w1))]
                emit_simjc(stA, 0, mid,
                           post=[lambda: tail_ic(pA, 0)] if pA is not None else None)  # P0
                if pA is not None:
                    tail_ic(pA, 1)
                mid = [lambda: tail_ic(pA, 2)] if pA is not None else []
                mid.append(lambda: stA.__setattr__("v", emit_v(w0, stA.xt)))
                emit_simjc(stA, 1, mid)               # P1
                stB = new_state(w1, nh0 if k == 0 else nh["B"])
                mid = [lambda: tail_transp(pA)] if pA is not None else []
                if k + 1 < W // 2:
                    mid.append(lambda: nh.__setitem__("A2", emit_head(2 * k + 2)))
                emit_simjc(stB, 0, mid,
                           post=[lambda: tail_ic(pB, 0)] if pB is not None else None)  # P2
                if pA is not None:
                    tail_proj(pA)
                mid = [lambda: stB.__setattr__("v", emit_v(w1, stB.xt))]
                emit_simjc(stB, 1, mid,
                           post=[lambda: tail_ic(pB, 1)] if pB is not None else None)  # P3
                mid = [lambda: tail_ic(pB, 2)] if pB is not None else None
                emit_pairtail(stA, stB, mid,
                              post=[lambda: tail_transp(pB)] if pB is not None else None)  # P4
                if pB is not None:
                    tail_proj(pB)
                head = nh.get("A2")
                pA, pB = stA, stB

            # epilogue: interleave the two windows' tails (independent)
            for ic in range(3):
                avtA = ps_tail.tile([128, 1, 512], F32, tag="t", name="eavtA")
                tail_avden(pA, [ic], avtA)
                avtB = ps_sh.tile([128, 2, 512], F32, tag="s", name="eavtB")
                tail_avden(pB, [ic], avtB)
                tail_norm(pA, [ic], avtA)
                tail_norm(pB, [ic], avtB)
            tail_transp(pA)
            tail_transp(pB)
            tail_proj(pA)
            tail_proj(pB)

    nc.finalize()
    return nc


# ---------------------------------------------------------------------------
# Harness entry point: full inputs in, full output out. Shards the 256
# windows across 8 NeuronCores (32 each), runs the Bass kernel via
# run_bass_kernel_spmd, and reassembles the full output.
# ---------------------------------------------------------------------------
from concourse.bass_utils import run_bass_kernel_spmd

_NC_CACHE = {}


def _get_nc():
    if "nc" not in _NC_CACHE:
        _NC_CACHE["nc"] = build_kernel(W=NB // 8)
    return _NC_CACHE["nc"]


def kernel(x, w_qkv, w_out, bias_table):
    x = np.asarray(x, dtype=np.float32)
    w_qkv = np.asarray(w_qkv, dtype=np.float32)
    w_out = np.asarray(w_out, dtype=np.float32)
    bias_table = np.asarray(bias_table, dtype=np.float32)

    in_maps = host_prep(x, w_qkv, w_out, bias_table, n_cores=8)
    nc = _get_nc()
    res = run_bass_kernel_spmd(nc, in_maps, core_ids=list(range(8)))
    return host_assemble(res.results)

